# revision 4
# baseline (speedup 1.0000x reference)
"""ChunkEmbedding Trainium2 kernel.

Computation (see reference):
  chunk[n, :] = sum_l (w[n,l]/sum_l w[n,l]) * emb_table[input_ids[n,l], :]
  then scatter chunk rows into [B, T, D] at (map_ids[n], pos[n]+1), add
  CLS/SEP rows and build the mask.

Strategy:
  - Data-parallel over chunks: 4096 chunks -> 512 per core on 8 cores.
  - Per core: for each tile of 128 chunks, for each token l in 0..63,
    indirect-DMA-gather emb rows [128, 768] (partition = chunk) and
    fused multiply-accumulate on the vector engine:
        acc = x * w[:, l] + acc        (w is a per-partition scalar)
  - Weight normalization, final scatter into the padded [B, T, D] output,
    CLS/SEP and mask are trivial O(input/output-size) host-side prep on
    numpy (part of shard/unshard).
"""

import os

import numpy as np

N_CORES = 8
N_CHUNKS = 4096
CHUNK_LEN = 64
DIM = 768
VOCAB = 30522
CPC = N_CHUNKS // N_CORES  # chunks per core = 512
P = 128
N_TILES = CPC // P  # 4
CLS_IDX, SEP_IDX = 101, 102

# Pool depth for in-flight gather tiles (each is [128, 768] f32 = 3KB/partition)
GATHER_BUFS = int(os.environ.get("CHUNKEMB_GATHER_BUFS", "16"))

_cache = {}


def _ensure_axon_ntff_hook():
    """The image's antenv package lacks axon_hooks, so trn_boot's NTFF
    profile hook registration silently degraded. Synthesize the module and
    register the ctypes-based hook so run_bass_kernel_spmd(trace=True) can
    capture NTFF profiles through the axon terminal."""
    import sys
    import types

    try:
        from antenv.axon_hooks import get_axon_ntff_profile_hook  # noqa: F401

        return True
    except ImportError:
        pass
    try:
        import antenv
        from trn_agent_boot.trn_boot import _ntff_profile_via_ctypes

        hook = _ntff_profile_via_ctypes("/opt/axon/libaxon_pjrt.so")
        if hook is None:
            return False
        mod = types.ModuleType("antenv.axon_hooks")
        mod._hook = hook
        mod.set_axon_ntff_profile_hook = lambda h: setattr(mod, "_hook", h)
        mod.get_axon_ntff_profile_hook = lambda: mod._hook
        sys.modules["antenv.axon_hooks"] = mod
        antenv.axon_hooks = mod
        return True
    except Exception:
        return False


def _build_program():
    import concourse.bacc as bacc
    import concourse.bass as bass
    import concourse.mybir as mybir
    import concourse.tile as tile

    nc = bacc.Bacc(
        "TRN2", target_bir_lowering=False, debug=False, num_devices=N_CORES
    )
    emb = nc.dram_tensor(
        "emb", [VOCAB, DIM], mybir.dt.float32, kind="ExternalInput"
    ).ap()
    ids = nc.dram_tensor(
        "ids", [CPC, CHUNK_LEN], mybir.dt.int32, kind="ExternalInput"
    ).ap()
    wn = nc.dram_tensor(
        "wn", [CPC, CHUNK_LEN], mybir.dt.float32, kind="ExternalInput"
    ).ap()
    out = nc.dram_tensor(
        "out", [CPC, DIM], mybir.dt.float32, kind="ExternalOutput"
    ).ap()

    with tile.TileContext(nc) as tc:
        with (
            tc.tile_pool(name="xpool", bufs=GATHER_BUFS) as xpool,
            tc.tile_pool(name="meta", bufs=2) as metapool,
            tc.tile_pool(name="accp", bufs=2) as accpool,
        ):
            for t in range(N_TILES):
                ids_t = metapool.tile([P, CHUNK_LEN], mybir.dt.int32, tag="ids")
                wn_t = metapool.tile([P, CHUNK_LEN], mybir.dt.float32, tag="wn")
                nc.sync.dma_start(out=ids_t[:], in_=ids[t * P : (t + 1) * P, :])
                nc.sync.dma_start(out=wn_t[:], in_=wn[t * P : (t + 1) * P, :])
                acc = accpool.tile([P, DIM], mybir.dt.float32, tag="acc")
                for l in range(CHUNK_LEN):
                    x = xpool.tile([P, DIM], mybir.dt.float32, tag="x")
                    nc.gpsimd.indirect_dma_start(
                        out=x[:],
                        out_offset=None,
                        in_=emb[:],
                        in_offset=bass.IndirectOffsetOnAxis(
                            ap=ids_t[:, l : l + 1], axis=0
                        ),
                    )
                    if l == 0:
                        nc.vector.tensor_scalar_mul(acc[:], x[:], wn_t[:, 0:1])
                    else:
                        nc.vector.scalar_tensor_tensor(
                            out=acc[:],
                            in0=x[:],
                            scalar=wn_t[:, l : l + 1],
                            in1=acc[:],
                            op0=mybir.AluOpType.mult,
                            op1=mybir.AluOpType.add,
                        )
                nc.sync.dma_start(out=out[t * P : (t + 1) * P, :], in_=acc[:])
    nc.compile()
    return nc


def _get_program():
    if "nc" not in _cache:
        _cache["nc"] = _build_program()
    return _cache["nc"]


def kernel(input_ids, kp_token_weights, map_ids, emb_table, batch_size, max_map_len):
    from concourse.bass_utils import run_bass_kernel_spmd

    ids = np.ascontiguousarray(np.asarray(input_ids, dtype=np.int32))
    w = np.asarray(kp_token_weights, dtype=np.float32)
    wn = np.ascontiguousarray(w / w.sum(axis=1, keepdims=True))
    emb = np.ascontiguousarray(np.asarray(emb_table, dtype=np.float32))
    assert ids.shape == (N_CHUNKS, CHUNK_LEN) and emb.shape == (VOCAB, DIM)

    nc = _get_program()
    in_maps = [
        {
            "emb": emb,
            "ids": ids[k * CPC : (k + 1) * CPC],
            "wn": wn[k * CPC : (k + 1) * CPC],
        }
        for k in range(N_CORES)
    ]
    trace = bool(int(os.environ.get("CHUNKEMB_TRACE", "0")))
    if trace:
        trace = _ensure_axon_ntff_hook()
    res = run_bass_kernel_spmd(
        nc,
        in_maps,
        core_ids=list(range(N_CORES)),
        trace=trace,
        trace_cores=list(range(N_CORES)) if trace else None,
    )
    _cache["last_results"] = res
    chunk = np.concatenate([r["out"] for r in res.results], axis=0)  # [4096, 768]

    # Host-side unshard: scatter chunk rows into the padded output.
    mi = np.asarray(map_ids, dtype=np.int64)
    B = int(batch_size)
    T = int(max_map_len) + 2
    counts = np.bincount(mi, minlength=B)
    offsets = np.cumsum(counts) - counts
    pos = np.arange(mi.shape[0], dtype=np.int64) - offsets[mi]
    ret = np.zeros((B, T, DIM), dtype=np.float32)
    ret[mi, pos + 1] = chunk
    ret[:, 0, :] = emb[CLS_IDX]
    ret[np.arange(B), counts + 1] = emb[SEP_IDX]
    mask = np.zeros((B, T), dtype=np.float32)
    mask[mi, pos + 1] = 1.0
    mask[:, 0] = 1.0
    mask[np.arange(B), counts + 1] = 1.0
    return ret, mask
